# revision 55
# baseline (speedup 1.0000x reference)
"""Trainium2 Bass kernel for nn_AttentionBlock (B=8, H=W=32, C=512, 8 heads).

Strategy: data-parallel over batch -- each of the 8 NeuronCores processes one
batch element end-to-end (no collectives).  Per core:

  x [T=1024, C=512] -> qkv -> per-head attention (T x T softmax) -> out proj.

v2 design (ACT-bound pipeline, ~125us HW vs 187us fp32r baseline):
  * all matmul data is bf16 (host pre-casts); PSUM accumulation stays fp32.
    Rel err ~8e-3 vs the 2e-2 gate.
  * S^T = k^T q uses K=64 ROW-TILED matmuls: head 2p reads SBUF partitions
    0:64, head 2p+1 partitions 64:128 (kT2 tiles store head pairs in
    exactly this layout, so no zero-padding is needed).
  * softmax exp runs on ScalarE (the only exp engine) in N=1024 calls,
    back-to-back for ~92us; ALL other PE work (QKV projection fillers,
    prev-pair PV half-chains, normalize steps) is paced into the PE gaps
    between exp calls via a payload queue, which also keeps the PE busy
    enough that the HAM clock never re-throttles to 1.2 GHz mid-kernel.
  * PSUM budget: 2x S^T slot pairs (4 banks) + 4 PV/filler banks; pair 3's
    PV joins the payload queue so the tail stays warm; its normalize
    multiplies overlap the out-projection's cc0-2 partial accumulations.
  * DMAs are batched (~0.6us issue cost each) and ordered critical-first
    per HWDGE ring: x quarters + pair-0 q/k weight columns, then bulk q/k,
    then v columns / out-proj weights (HBM bw is shared by all 8 cores).
  * denominators come free from a ones-column appended to V (PV row 64);
    normalization = DVE multiply by a GPSIMD-broadcast reciprocal, split
    into recip and multiply payload steps so two heads' chains overlap.
  * out-projection uses anorm tiles as lhsT so the output lands directly in
    [t, c] layout -- no output transpose.  Output is bf16; host casts back.
  * no max-subtraction: logits are ~N(0,1) by construction (1/8 scale is
    folded into the ScalarE exp activation).

HW-behavior notes learned the hard way (see git-less history):
  * reciprocal_approx_fast reading PSUM returns garbage on HW (sim is
    fine) -- copy denominators to SBUF first.
  * mixing tile sizes (K=64 row-tiled + K=128) within one PSUM
    accumulation group hangs the device (NRT_EXEC_UNIT_UNRECOVERABLE).
  * DVE tensor_tensor cannot read two PSUM operands (NCC_IBVF027).
  * matmul PSUM output must be fp32 on TRN2 (bf16 out is TRN3+), so
    moving operands are capped at N=512 per PSUM bank.
  * tile pools release in LIFO order per side; use side="right" for
    transient pools that outlive/interleave the left stack.
"""

import math
import os
from contextlib import ExitStack

import numpy as np

import concourse.bass as bass
import concourse.mybir as mybir
import concourse.tile as tile
from concourse import bacc

T = 1024          # tokens per batch element (32*32)
C = 512           # channels
HEADS = 8
HC = C // HEADS   # 64
P = 128           # partitions
NT = T // P       # 8 t-tiles (also 8 s-tiles)
NCT = C // P      # 4 c-tiles
CHUNK = 512       # matmul moving-operand chunk (one fp32 PSUM bank)
NCH = T // CHUNK  # 2 chunks
NPAIR = HEADS // 2
F32 = mybir.dt.float32
BF16 = mybir.dt.bfloat16
EXP_SCALE = 1.0 / math.sqrt(HC)  # (1/sqrt(sqrt(hc)))^2 applied to q.k
VSTRIDE = HC + 1  # 65: v columns + ones column per head
VAW = HEADS * VSTRIDE + (P - VSTRIDE)  # PV lhsT 128-wide reads stay in-tile


def build_program(debug_dumps=False):
    nc = bacc.Bacc("TRN2", num_devices=8, debug=False)

    x_d = nc.dram_tensor("x", [T, C], BF16, kind="ExternalInput")
    wqkv_d = nc.dram_tensor("qkv_w", [C, 3 * C], BF16, kind="ExternalInput")
    wout_d = nc.dram_tensor("out_w", [C, C], BF16, kind="ExternalInput")
    qkb_d = nc.dram_tensor("qk_b", [2 * C], F32, kind="ExternalInput")
    ob_d = nc.dram_tensor("out_b", [C], F32, kind="ExternalInput")
    out_d = nc.dram_tensor("out", [T, C], BF16, kind="ExternalOutput")
    dbg = {}
    if debug_dumps:
        for nm, shp in [
            ("dbg_xT", [P, NCT * T]), ("dbg_q0", [P, T]), ("dbg_k0", [P, T]),
            ("dbg_exh0", [P, 2 * T]), ("dbg_va0", [P, VAW]),
            ("dbg_an0", [P, T]),
        ]:
            dbg[nm] = nc.dram_tensor(nm, shp, BF16, kind="ExternalOutput")

    with tile.TileContext(nc) as tc, ExitStack() as ctx:
        from concourse.masks import make_identity

        # ---------------- SBUF pools ----------------
        const = ctx.enter_context(tc.tile_pool(name="const", bufs=1))
        persist = ctx.enter_context(tc.tile_pool(name="persist", bufs=1))
        workp = ctx.enter_context(tc.tile_pool(name="workp", bufs=1))

        # x in FOUR batched DMAs (2 per HWDGE queue) so the first quarter
        # lands ASAP; per-DMA issue cost is ~0.6us.  Layout:
        # x_in[:, i*C + c] = x[i*128 + p, c] (t-tile-major).
        x_in = persist.tile([P, NT * C], BF16, tag="x_in", name="x_in")
        xr = x_d.ap().rearrange("(i p) c -> p i c", p=P)  # [128, 8, 512]
        xv = x_in[:].rearrange("p (i c) -> p i c", i=NT)
        nc.sync.dma_start(xv[:, 0:2, :], xr[:, 0:2, :])
        nc.scalar.dma_start(xv[:, 2:4, :], xr[:, 2:4, :])
        nc.sync.dma_start(xv[:, 4:6, :], xr[:, 4:6, :])
        nc.scalar.dma_start(xv[:, 6:8, :], xr[:, 6:8, :])

        identity = const.tile([P, P], BF16, tag="ident", name="ident")
        make_identity(nc, identity[:])
        warm_rhs = const.tile([P, CHUNK], BF16, tag="warm", name="warm_rhs")
        nc.gpsimd.memset(warm_rhs[:], 0.0)
        # ones1: K=1 all-ones lhsT for the tail's PE-broadcast normalize
        ones1 = const.tile([1, P], F32, tag="ones1", name="ones1")
        nc.gpsimd.memset(ones1[:], 1.0)

        # pair-0's q/k weight columns as two tiny DMAs that land first,
        # so the first S^T slot starts before the bulk weights arrive
        wq_eq = persist.tile([P, NCT * P], BF16, tag="wq_eq", name="wq_eq")
        wq_ek = persist.tile([P, NCT * P], BF16, tag="wq_ek", name="wq_ek")
        wqr = wqkv_d.ap().rearrange("(m p) c -> p m c", p=P)  # [128,4,1536]
        nc.sync.dma_start(
            wq_eq[:].rearrange("p (m c) -> p m c", m=NCT), wqr[:, :, 0:P])
        nc.scalar.dma_start(
            wq_ek[:].rearrange("p (m c) -> p m c", m=NCT),
            wqr[:, :, C:C + P])

        # qkv weights: q/k columns first (they gate the qk projections),
        # v columns on the slower SWDGE queue afterwards
        wq = []  # [c-tile][128, 1536] bf16
        for m in range(NCT):
            t_ = persist.tile([P, 3 * C], BF16, tag=f"wq{m}", name=f"wq{m}")
            eng = nc.sync if m % 2 == 0 else nc.scalar
            eng.dma_start(t_[:, 0:2 * C],
                          wqkv_d.ap()[m * P:(m + 1) * P, 0:2 * C])
            wq.append(t_)
        # v columns + out-proj weights queue BEHIND the critical x/q/k
        # transfers on the same HWDGE rings (each ring drains in FIFO order,
        # so critical data gets the HBM bandwidth first)
        for m in range(NCT):
            eng = nc.sync if m % 2 == 0 else nc.scalar
            eng.dma_start(wq[m][:, 2 * C:3 * C],
                          wqkv_d.ap()[m * P:(m + 1) * P, 2 * C:3 * C])

        # bias tiles; column m = qk_b[128m:128m+128]
        qkb_all = const.tile([P, 2 * C // P], F32, tag="qkball", name="qkb_all")
        nc.gpsimd.dma_start(
            qkb_all[:], qkb_d.ap().rearrange("(m p) -> p m", p=P)
        )
        qkb_t = [qkb_all[:, m:m + 1] for m in range(2 * C // P)]
        ob_row = const.tile([1, C], F32, tag="obrow", name="ob_row")
        nc.gpsimd.dma_start(ob_row[:], ob_d.ap().rearrange("(o c) -> o c", o=1))
        ob_bcast = const.tile([P, C], F32, tag="obb", name="ob_bcast")
        nc.gpsimd.partition_broadcast(ob_bcast[:], ob_row[:], channels=P)

        # out-proj weights: single batched DMA, needed only in phase 3
        wo_all = persist.tile([P, NCT * C], BF16, tag="wo", name="wo_all")
        nc.sync.dma_start(
            wo_all[:].rearrange("p (m c) -> p m c", m=NCT),
            wout_d.ap().rearrange("(m p) c -> p m c", p=P),
        )
        wo = [wo_all[:, m * C:(m + 1) * C] for m in range(NCT)]

        # persistent activation tiles
        # xT_all[:, cc*T + t] = x^T tile cc: [c-within-tile, t]
        xT_all = persist.tile([P, NCT * T], BF16, tag="xT", name="xT_all")
        qkT = [persist.tile([P, T], BF16, tag=f"qk{m}", name=f"qk{m}")
               for m in range(NCT)]
        # kT2[m]: rows 0:64 = k^T head 2m, rows 64:128 = k^T head 2m+1
        kT2 = [persist.tile([P, T], BF16, tag=f"k2{m}", name=f"k2{m}")
               for m in range(NCT)]
        vaug = [persist.tile([P, VAW], BF16, tag=f"va{i}", name=f"va{i}")
                for i in range(NT)]
        for i in range(NT):
            nc.gpsimd.memset(vaug[i][:], 1.0)  # ones column (+padding) preset
        anorm = [persist.tile([P, T], BF16, tag=f"an{m}", name=f"an{m}")
                 for m in range(NCT)]

        # ================= prologue =================
        # HAM warm-up: real (non-transpose) matmuls on junk data while the x
        # DMA lands, so the PE clock is at 8/8 when the real work starts.
        with tc.tile_pool(name="ps_warm", bufs=1, space="PSUM") as ps_warm:
            ps_w = ps_warm.tile([P, CHUNK], F32, tag="w", name="ps_w")
            for _ in range(5):
                nc.tensor.matmul(ps_w[:], identity[:], warm_rhs[:],
                                 start=True, stop=True)

        # exp ACT-table preload: a tiny dummy exp during the DMA wait pays the
        # ~2.7us one-time table-load cost before the real pipeline needs it.
        scratch16 = workp.tile([1, 16], F32, tag="scr16", name="scratch16")
        nc.scalar.activation(
            scratch16[:], warm_rhs[0:1, 0:16],
            mybir.ActivationFunctionType.Exp, scale=1.0)



        pv_cm = tc.tile_pool(name="ps_pv", bufs=1, space="PSUM", side="right")
        state_pv_pool = pv_cm.__enter__()
        tr_cm = tc.tile_pool(name="ps_tr", bufs=2, space="PSUM", side="right")
        tr_pool = tr_cm.__enter__()

        def emit_transpose(i):
            # x PE transpose; xT_all[:, cc*T + i*128 : ...] gets tile (i, cc)
            ps_tr = tr_pool.tile([P, C], BF16, tag="tr", name="ps_tr")
            for cc in range(NCT):
                nc.tensor.transpose(
                    ps_tr[:, cc * P:(cc + 1) * P],
                    x_in[:, i * C + cc * P: i * C + (cc + 1) * P],
                    identity[:],
                )
            # one strided copy: dest [128, cc, 128] with cc-stride T.
            # Tiles 4-7 copy on the (still idle) ScalarE to unload DVE.
            dst = xT_all[:].rearrange("p (cc t) -> p cc t", cc=NCT)
            if i < NT // 2:
                nc.vector.tensor_copy(
                    dst[:, :, i * P:(i + 1) * P],
                    ps_tr[:].rearrange("p (cc q) -> p cc q", cc=NCT),
                )
            else:
                nc.scalar.copy(
                    dst[:, :, i * P:(i + 1) * P],
                    ps_tr[:].rearrange("p (cc q) -> p cc q", cc=NCT),
                )

        qk_tag = [0]

        def qk_psum():
            # rotate across all four pv banks: they are idle until pair 1,
            # and a 4-deep rotation hides the DVE bias-add WAR latency
            qk_tag[0] = (qk_tag[0] + 1) % 4
            return state_pv_pool.tile(
                [P, CHUNK], F32, tag=f"pv{qk_tag[0]}", name="ps_qk")

        def emit_qk(m, j):
            ps_qk = qk_psum()
            js = slice(j * CHUNK, (j + 1) * CHUNK)
            for cc in range(NCT):
                if m == 0:
                    lhsT = wq_eq[:, cc * P:(cc + 1) * P]
                elif m == NCT:
                    lhsT = wq_ek[:, cc * P:(cc + 1) * P]
                else:
                    lhsT = wq[cc][:, m * P:(m + 1) * P]
                nc.tensor.matmul(
                    ps_qk[:],
                    lhsT,
                    xT_all[:, cc * T + j * CHUNK: cc * T + (j + 1) * CHUNK],
                    start=(cc == 0),
                    stop=(cc == NCT - 1),
                )
            dstt = qkT[m] if m < NCT else kT2[m - NCT]
            nc.vector.tensor_scalar_add(dstt[:, js], ps_qk[:], qkb_t[m][:])

        def emit_v(i):
            ps_v = qk_psum()
            for cc in range(NCT):
                nc.tensor.matmul(
                    ps_v[:],
                    xT_all[:, cc * T + i * P: cc * T + (i + 1) * P],
                    wq[cc][:, 2 * C:3 * C],
                    start=(cc == 0),
                    stop=(cc == NCT - 1),
                )
            va3 = vaug[i][:, 0:HEADS * VSTRIDE].rearrange(
                "p (h d) -> p h d", d=VSTRIDE)
            nc.vector.tensor_copy(
                va3[:, :, 0:HC],
                ps_v[:].rearrange("p (h d) -> p h d", h=HEADS),
            )

        # prologue part 2: transposes interleaved with pair 0's q/k tiles so
        # the first S^T slot is reachable as early as possible.  Chunk j of
        # qkT[0]/kT2[0] needs x tiles 4j..4j+3 transposed.
        for i in range(NT // 2):
            emit_transpose(i)
        emit_qk(0, 0)
        emit_qk(NCT, 0)
        for i in range(NT // 2, NT):
            emit_transpose(i)
        emit_qk(0, 1)
        emit_qk(NCT, 1)
        tr_cm.__exit__(None, None, None)

        # filler iterator: remaining phase-1 work, just-in-time order --
        # pair 1 needs (m1,k1) + all of v (for PV of pair 0); m2/k2 and
        # m3/k3 can wait until pairs 1-2, lightening pair 0's slots
        def phase1_fillers():
            for m in (1, NCT + 1):
                for j in range(NCH):
                    yield ("qk", m, j)
            for i in range(NT):
                yield ("v", i)
            for m in (2, NCT + 2, 3, NCT + 3):
                for j in range(NCH):
                    yield ("qk", m, j)

        fillers = phase1_fillers()
        state = {"fill_done": False, "tail_boxes": []}

        def run_fillers(n):
            for _ in range(n):
                try:
                    f = next(fillers)
                except StopIteration:
                    state["fill_done"] = True
                    return
                if f[0] == "qk":
                    emit_qk(f[1], f[2])
                else:
                    emit_v(f[1])

        # ================= phase 2: attention =================
        def emit_pv_chain(h, j, ppv, exh):
            base = (h % 2) * T
            for ssi in range(NT):
                nc.tensor.matmul(
                    ppv[:],
                    vaug[ssi][:, h * VSTRIDE: h * VSTRIDE + P],
                    exh[:, ssi * 2 * T + base + j * CHUNK:
                        ssi * 2 * T + base + (j + 1) * CHUNK],
                    start=(ssi == 0),
                    stop=(ssi == NT - 1),
                )

        def emit_recip(h, box, pe_pool=None):
            hh = h % 2
            dtmp = workp.tile([1, T], F32, tag=f"dtmp{hh}", name="dtmp")
            nc.vector.tensor_copy(dtmp[:, 0:CHUNK], box["p0"][HC:HC + 1, :])
            nc.vector.tensor_copy(dtmp[:, CHUNK:T], box["p1"][HC:HC + 1, :])
            recip = workp.tile([1, T], F32, tag=f"recip{hh}", name="recip")
            nc.vector.reciprocal_approx_fast(recip[:], dtmp[:])
            bcast = workp.tile([HC, T], F32, tag=f"bcast{hh}", name="bcast")
            if pe_pool is None:
                nc.gpsimd.partition_broadcast(bcast[:], recip[:], channels=HC)
            else:
                # PE broadcast + DVE copy: keeps GPSIMD (and its drains) off
                # the tail critical path, and the matmuls keep HAM warm
                bc_ps = pe_pool.tile([HC, T], F32, tag="bc", name="bc_ps")
                for j in range(NCH):
                    js = slice(j * CHUNK, (j + 1) * CHUNK)
                    nc.tensor.matmul(bc_ps[0:HC, js], ones1[:, 0:HC],
                                     recip[:, js], start=True, stop=True)
                nc.vector.tensor_copy(bcast[:], bc_ps[:])
            box["bc"] = bcast

        def emit_mults(h, box, js=(0, 1)):
            m = h // 2
            rlo = (h % 2) * HC
            for j in js:
                ppv = box["p0"] if j == 0 else box["p1"]
                nc.vector.tensor_tensor(
                    anorm[m][rlo:rlo + HC, j * CHUNK:(j + 1) * CHUNK],
                    ppv[0:HC, :],
                    box["bc"][:, j * CHUNK:(j + 1) * CHUNK],
                    op=mybir.AluOpType.mult,
                )

        def emit_pv_half(h, j, ppv, exh, lo):
            base = (h % 2) * T
            for ssi in range(lo, lo + NT // 2):
                nc.tensor.matmul(
                    ppv[:],
                    vaug[ssi][:, h * VSTRIDE: h * VSTRIDE + P],
                    exh[:, ssi * 2 * T + base + j * CHUNK:
                        ssi * 2 * T + base + (j + 1) * CHUNK],
                    start=(ssi == 0),
                    stop=(ssi == NT - 1),
                )

        def make_pv_steps(p, exh, tail=False):
            """Twelve ~0.9us payload units per pair: per head, each PV chunk
            chain is split into two 4-matmul halves plus a reciprocal step;
            both heads' normalize-multiplies come last (recips overlap)."""
            steps = []
            tail_mults = []
            for hh in range(2):
                h = 2 * p + hh
                box = {}

                def s_alloc0(h=h, hh=hh, box=box):
                    box["p0"] = state_pv_pool.tile(
                        [P, CHUNK], F32, tag=f"pv{2 * hh}", name="ppv0")
                    emit_pv_half(h, 0, box["p0"], exh, 0)

                def s_j0b(h=h, box=box):
                    emit_pv_half(h, 0, box["p0"], exh, NT // 2)

                def s_j1a(h=h, hh=hh, box=box):
                    box["p1"] = state_pv_pool.tile(
                        [P, CHUNK], F32, tag=f"pv{2 * hh + 1}", name="ppv1")
                    emit_pv_half(h, 1, box["p1"], exh, 0)

                def s_j1b(h=h, box=box):
                    emit_pv_half(h, 1, box["p1"], exh, NT // 2)

                def s_recip(h=h, box=box):
                    emit_recip(h, box)

                def s_mults(h=h, box=box):
                    emit_mults(h, box)

                steps += [s_alloc0, s_j0b, s_j1a, s_j1b, s_recip]
                if tail:
                    state["tail_boxes"].append(box)
                else:
                    tail_mults.append(s_mults)
            steps += tail_mults
            return steps

        exh_pool = ctx.enter_context(tc.tile_pool(name="exh", bufs=3))

        # right stack: opened after ps_tr closed, outlives the (left) qk pool
        st_cm = tc.tile_pool(name="ps_st", bufs=2, space="PSUM", side="right")
        st_pool = st_cm.__enter__()

        slots = [(p, ssi) for p in range(NPAIR) for ssi in range(NT)]
        exhs = []
        st_q = []

        def ensure_exh(p):
            while len(exhs) <= p:
                exhs.append(exh_pool.tile(
                    [P, NT * 2 * T], BF16, tag="exh", name="exh"))

        def emit_st(p, ssi):
            ensure_exh(p)
            sta = st_pool.tile([P, T], F32, tag="st", name="sta")
            stb = st_pool.tile([P, T], F32, tag="st", name="stb")
            for j in range(NCH):
                js = slice(j * CHUNK, (j + 1) * CHUNK)
                nc.tensor.matmul(
                    sta[:, js],
                    kT2[p][0:HC, ssi * P:(ssi + 1) * P],
                    qkT[p][0:HC, js],
                    start=True, stop=True,
                )
                nc.tensor.matmul(
                    stb[:, js],
                    kT2[p][HC:P, ssi * P:(ssi + 1) * P],
                    qkT[p][HC:P, js],
                    start=True, stop=True,
                )
            st_q.append((sta, stb))

        pending = []  # queue of PV/normalize closures for the previous pair
        emit_st(*slots[0])
        for g, (p, ssi) in enumerate(slots):
            exh = exhs[p]
            sta, stb = st_q.pop(0)
            nc.scalar.activation(
                exh[:, ssi * 2 * T: ssi * 2 * T + T],
                sta[:],
                mybir.ActivationFunctionType.Exp,
                scale=EXP_SCALE,
            )
            nc.scalar.activation(
                exh[:, ssi * 2 * T + T: (ssi + 1) * 2 * T],
                stb[:],
                mybir.ActivationFunctionType.Exp,
                scale=EXP_SCALE,
            )
            # next slot's S^T goes in front of this slot's payload work
            if g + 1 < len(slots):
                emit_st(*slots[g + 1])
            if ssi == 0 and p > 0:
                pending.extend(make_pv_steps(p - 1, exhs[p - 1]))
                if debug_dumps and p == 1:
                    nc.sync.dma_start(
                        dbg["dbg_exh0"].ap(), exhs[0][:, 0:2 * T])
            if ssi == 4 and p == NPAIR - 1:
                # pair 3's own PV joins the payload queue (its early-s-tile
                # halves only need already-finished exps); keeps the PE warm
                # through the last slots and shrinks the tail
                pending.extend(make_pv_steps(p, exh, tail=True))
            # PE-gap payloads for this slot
            if not state["fill_done"]:
                run_fillers(2)
            if state["fill_done"]:
                npop = 2 if len(pending) >= 6 else 1
                for _ in range(npop):
                    if pending:
                        pending.pop(0)()
        # drain everything (pair 3's normalize is handled below, woven
        # into the out-projection)
        while pending:
            pending.pop(0)()
        st_cm.__exit__(None, None, None)

        if debug_dumps:
            nc.sync.dma_start(dbg["dbg_xT"].ap(), xT_all[:])
            nc.sync.dma_start(dbg["dbg_q0"].ap(), qkT[0][:])
            nc.sync.dma_start(dbg["dbg_k0"].ap(), kT2[0][:])
            nc.sync.dma_start(dbg["dbg_va0"].ap(), vaug[0][:])

        # ================= phase 3: out projection =================
        # pair 3's normalize runs interleaved: PE-broadcast recips, then the
        # j0 mults (which unblock t-tiles 0..3), then the j1 mults.
        ps_o_cm = tc.tile_pool(name="ps_o", bufs=3, space="PSUM")
        ps_op = ps_o_cm.__enter__()
        box_a, box_b = state["tail_boxes"]
        otiles = []
        for i in range(3):
            ps_o = ps_op.tile([P, C], F32, tag="o", name="ps_o")
            otiles.append(ps_o)
            for cc in range(NCT - 1):
                nc.tensor.matmul(
                    ps_o[:],
                    anorm[cc][:, i * P:(i + 1) * P],
                    wo[cc][:],
                    start=(cc == 0),
                    stop=False,
                )
        emit_mults(2 * (NPAIR - 1), box_a, js=(0,))
        emit_mults(2 * (NPAIR - 1) + 1, box_b, js=(0,))
        emit_mults(2 * (NPAIR - 1), box_a, js=(1,))
        emit_mults(2 * (NPAIR - 1) + 1, box_b, js=(1,))

        def finish_tile(i, ps_o):
            nc.tensor.matmul(
                ps_o[:],
                anorm[NCT - 1][:, i * P:(i + 1) * P],
                wo[NCT - 1][:],
                start=False,
                stop=True,
            )
            osb = workp.tile([P, C], BF16, tag=f"osb{i}", name=f"osb{i}")
            nc.vector.tensor_tensor(
                osb[:], ps_o[:], ob_bcast[:], op=mybir.AluOpType.add)
            nc.sync.dma_start(out_d.ap()[i * P:(i + 1) * P, :], osb[:])

        for i in range(3):
            finish_tile(i, otiles[i])
        for i in range(3, NT):
            ps_o = ps_op.tile([P, C], F32, tag="o", name="ps_o")
            for cc in range(NCT - 1):
                nc.tensor.matmul(
                    ps_o[:],
                    anorm[cc][:, i * P:(i + 1) * P],
                    wo[cc][:],
                    start=(cc == 0),
                    stop=False,
                )
            finish_tile(i, ps_o)
        ps_o_cm.__exit__(None, None, None)
        pv_cm.__exit__(None, None, None)
        if debug_dumps:
            nc.sync.dma_start(dbg["dbg_an0"].ap(), anorm[0][:])

    nc.compile()
    return nc


_CACHED_NC = None


def _get_nc():
    global _CACHED_NC
    if _CACHED_NC is None:
        _CACHED_NC = build_program(
            debug_dumps=bool(int(os.environ.get("KERNEL_DEBUG", "0"))))
    return _CACHED_NC


def _prep_inputs(x, qkv_w, qkv_b, out_w, out_b):
    import ml_dtypes

    x = np.asarray(x)
    B = x.shape[0]
    x2 = x.reshape(B, T, C).astype(ml_dtypes.bfloat16)
    wq2 = np.asarray(qkv_w).reshape(C, 3 * C).astype(ml_dtypes.bfloat16)
    wo2 = np.asarray(out_w).reshape(C, C).astype(ml_dtypes.bfloat16)
    qkv_b = np.asarray(qkv_b).astype(np.float32)
    out_b = np.asarray(out_b).astype(np.float32)
    # fold the v-bias through the output projection (exact: A_norm += b_v
    # shifts out by b_v @ W_out since softmax rows sum to 1).
    b_v = qkv_b[2 * C:3 * C]
    ob_eff = (
        out_b.astype(np.float64)
        + b_v.astype(np.float64) @ wo2.astype(np.float64)
    ).astype(np.float32)
    qkb = np.ascontiguousarray(qkv_b[0:2 * C])
    return x2, wq2, wo2, qkb, ob_eff


def kernel(x, qkv_w, qkv_b, out_w, out_b):
    """Full inputs in, full output out.  Shards batch across 8 NeuronCores."""
    from concourse.bass_utils import run_bass_kernel_spmd

    x = np.asarray(x)
    B, H, W, Cc = x.shape
    assert (B, H, W, Cc) == (8, 32, 32, C)
    x2, wq2, wo2, qkb, ob_eff = _prep_inputs(x, qkv_w, qkv_b, out_w, out_b)

    nc = _get_nc()
    in_maps = [
        {
            "x": np.ascontiguousarray(x2[b]),
            "qkv_w": np.ascontiguousarray(wq2),
            "out_w": np.ascontiguousarray(wo2),
            "qk_b": qkb,
            "out_b": ob_eff,
        }
        for b in range(B)
    ]
    trace = bool(int(os.environ.get("KERNEL_TRACE", "0")))
    res = run_bass_kernel_spmd(nc, in_maps, core_ids=list(range(B)), trace=trace)
    if trace and res.exec_time_ns is not None:
        print(f"HW exec time: {res.exec_time_ns} ns")
    kernel.last_results = res
    out = np.stack(
        [np.asarray(res.results[b]["out"]).astype(np.float32) for b in range(B)],
        axis=0,
    )
    return out.reshape(B, H, W, Cc)


kernel.last_results = None


# revision 56
# speedup vs baseline: 1.0266x; 1.0266x over previous
"""Trainium2 Bass kernel for nn_AttentionBlock (B=8, H=W=32, C=512, 8 heads).

Strategy: data-parallel over batch -- each of the 8 NeuronCores processes one
batch element end-to-end (no collectives).  Per core:

  x [T=1024, C=512] -> qkv -> per-head attention (T x T softmax) -> out proj.

v2 design (ACT-bound pipeline, ~125us HW vs 187us fp32r baseline):
  * all matmul data is bf16 (host pre-casts); PSUM accumulation stays fp32.
    Rel err ~8e-3 vs the 2e-2 gate.
  * S^T = k^T q uses K=64 ROW-TILED matmuls: head 2p reads SBUF partitions
    0:64, head 2p+1 partitions 64:128 (kT2 tiles store head pairs in
    exactly this layout, so no zero-padding is needed).
  * softmax exp runs on ScalarE (the only exp engine) in N=1024 calls,
    back-to-back for ~92us; ALL other PE work (QKV projection fillers,
    prev-pair PV half-chains, normalize steps) is paced into the PE gaps
    between exp calls via a payload queue, which also keeps the PE busy
    enough that the HAM clock never re-throttles to 1.2 GHz mid-kernel.
  * PSUM budget: 2x S^T slot pairs (4 banks) + 4 PV/filler banks; pair 3's
    PV joins the payload queue so the tail stays warm; its normalize
    multiplies overlap the out-projection's cc0-2 partial accumulations.
  * DMAs are batched (~0.6us issue cost each) and ordered critical-first
    per HWDGE ring: x quarters + pair-0 q/k weight columns, then bulk q/k,
    then v columns / out-proj weights (HBM bw is shared by all 8 cores).
  * denominators come free from a ones-column appended to V (PV row 64);
    normalization = DVE multiply by a GPSIMD-broadcast reciprocal, split
    into recip and multiply payload steps so two heads' chains overlap.
  * out-projection uses anorm tiles as lhsT so the output lands directly in
    [t, c] layout -- no output transpose.  Output is bf16; host casts back.
  * no max-subtraction: logits are ~N(0,1) by construction (1/8 scale is
    folded into the ScalarE exp activation).

HW-behavior notes learned the hard way (see git-less history):
  * reciprocal_approx_fast reading PSUM returns garbage on HW (sim is
    fine) -- copy denominators to SBUF first.
  * mixing tile sizes (K=64 row-tiled + K=128) within one PSUM
    accumulation group hangs the device (NRT_EXEC_UNIT_UNRECOVERABLE).
  * DVE tensor_tensor cannot read two PSUM operands (NCC_IBVF027).
  * matmul PSUM output must be fp32 on TRN2 (bf16 out is TRN3+), so
    moving operands are capped at N=512 per PSUM bank.
  * tile pools release in LIFO order per side; use side="right" for
    transient pools that outlive/interleave the left stack.
"""

import math
import os
from contextlib import ExitStack

import numpy as np

import concourse.bass as bass
import concourse.mybir as mybir
import concourse.tile as tile
from concourse import bacc

T = 1024          # tokens per batch element (32*32)
C = 512           # channels
HEADS = 8
HC = C // HEADS   # 64
P = 128           # partitions
NT = T // P       # 8 t-tiles (also 8 s-tiles)
NCT = C // P      # 4 c-tiles
CHUNK = 512       # matmul moving-operand chunk (one fp32 PSUM bank)
NCH = T // CHUNK  # 2 chunks
NPAIR = HEADS // 2
F32 = mybir.dt.float32
BF16 = mybir.dt.bfloat16
EXP_SCALE = 1.0 / math.sqrt(HC)  # (1/sqrt(sqrt(hc)))^2 applied to q.k
VSTRIDE = HC + 1  # 65: v columns + ones column per head
VAW = HEADS * VSTRIDE + (P - VSTRIDE)  # PV lhsT 128-wide reads stay in-tile


def build_program(debug_dumps=False):
    nc = bacc.Bacc("TRN2", num_devices=8, debug=False)

    x_d = nc.dram_tensor("x", [T, C], BF16, kind="ExternalInput")
    wqkv_d = nc.dram_tensor("qkv_w", [C, 3 * C], BF16, kind="ExternalInput")
    wout_d = nc.dram_tensor("out_w", [C, C], BF16, kind="ExternalInput")
    qkb_d = nc.dram_tensor("qk_b", [2 * C], F32, kind="ExternalInput")
    ob_d = nc.dram_tensor("out_b", [C], F32, kind="ExternalInput")
    out_d = nc.dram_tensor("out", [T, C], BF16, kind="ExternalOutput")
    dbg = {}
    if debug_dumps:
        for nm, shp in [
            ("dbg_xT", [P, NCT * T]), ("dbg_q0", [P, T]), ("dbg_k0", [P, T]),
            ("dbg_exh0", [P, 2 * T]), ("dbg_va0", [P, VAW]),
            ("dbg_an0", [P, T]),
        ]:
            dbg[nm] = nc.dram_tensor(nm, shp, BF16, kind="ExternalOutput")

    with tile.TileContext(nc) as tc, ExitStack() as ctx:
        from concourse.masks import make_identity

        # ---------------- SBUF pools ----------------
        const = ctx.enter_context(tc.tile_pool(name="const", bufs=1))
        persist = ctx.enter_context(tc.tile_pool(name="persist", bufs=1))
        workp = ctx.enter_context(tc.tile_pool(name="workp", bufs=1))

        # x in FOUR batched DMAs (2 per HWDGE queue) so the first quarter
        # lands ASAP; per-DMA issue cost is ~0.6us.  Layout:
        # x_in[:, i*C + c] = x[i*128 + p, c] (t-tile-major).
        x_in = persist.tile([P, NT * C], BF16, tag="x_in", name="x_in")
        xr = x_d.ap().rearrange("(i p) c -> p i c", p=P)  # [128, 8, 512]
        xv = x_in[:].rearrange("p (i c) -> p i c", i=NT)
        nc.sync.dma_start(xv[:, 0:2, :], xr[:, 0:2, :])
        nc.scalar.dma_start(xv[:, 2:4, :], xr[:, 2:4, :])
        nc.sync.dma_start(xv[:, 4:6, :], xr[:, 4:6, :])
        nc.scalar.dma_start(xv[:, 6:8, :], xr[:, 6:8, :])

        identity = const.tile([P, P], BF16, tag="ident", name="ident")
        make_identity(nc, identity[:])
        warm_rhs = const.tile([P, CHUNK], BF16, tag="warm", name="warm_rhs")
        nc.gpsimd.memset(warm_rhs[:], 0.0)
        # ones1: K=1 all-ones lhsT for the tail's PE-broadcast normalize
        ones1 = const.tile([1, P], F32, tag="ones1", name="ones1")
        nc.gpsimd.memset(ones1[:], 1.0)

        # pair-0's q/k weight columns as two tiny DMAs that land first,
        # so the first S^T slot starts before the bulk weights arrive
        wq_eq = persist.tile([P, NCT * P], BF16, tag="wq_eq", name="wq_eq")
        wq_ek = persist.tile([P, NCT * P], BF16, tag="wq_ek", name="wq_ek")
        wqr = wqkv_d.ap().rearrange("(m p) c -> p m c", p=P)  # [128,4,1536]
        nc.sync.dma_start(
            wq_eq[:].rearrange("p (m c) -> p m c", m=NCT), wqr[:, :, 0:P])
        nc.scalar.dma_start(
            wq_ek[:].rearrange("p (m c) -> p m c", m=NCT),
            wqr[:, :, C:C + P])

        # qkv weights: q/k columns first (they gate the qk projections),
        # v columns on the slower SWDGE queue afterwards
        wq = []  # [c-tile][128, 1536] bf16
        for m in range(NCT):
            t_ = persist.tile([P, 3 * C], BF16, tag=f"wq{m}", name=f"wq{m}")
            eng = nc.sync if m % 2 == 0 else nc.scalar
            eng.dma_start(t_[:, 0:2 * C],
                          wqkv_d.ap()[m * P:(m + 1) * P, 0:2 * C])
            wq.append(t_)
        # v columns + out-proj weights queue BEHIND the critical x/q/k
        # transfers on the same HWDGE rings (each ring drains in FIFO order,
        # so critical data gets the HBM bandwidth first)
        for m in range(NCT):
            eng = nc.sync if m % 2 == 0 else nc.scalar
            eng.dma_start(wq[m][:, 2 * C:3 * C],
                          wqkv_d.ap()[m * P:(m + 1) * P, 2 * C:3 * C])

        # bias tiles; column m = qk_b[128m:128m+128]
        qkb_all = const.tile([P, 2 * C // P], F32, tag="qkball", name="qkb_all")
        nc.gpsimd.dma_start(
            qkb_all[:], qkb_d.ap().rearrange("(m p) -> p m", p=P)
        )
        qkb_t = [qkb_all[:, m:m + 1] for m in range(2 * C // P)]
        ob_row = const.tile([1, C], F32, tag="obrow", name="ob_row")
        nc.gpsimd.dma_start(ob_row[:], ob_d.ap().rearrange("(o c) -> o c", o=1))
        ob_bcast = const.tile([P, C], F32, tag="obb", name="ob_bcast")
        nc.gpsimd.partition_broadcast(ob_bcast[:], ob_row[:], channels=P)

        # out-proj weights: single batched DMA, needed only in phase 3
        wo_all = persist.tile([P, NCT * C], BF16, tag="wo", name="wo_all")
        nc.sync.dma_start(
            wo_all[:].rearrange("p (m c) -> p m c", m=NCT),
            wout_d.ap().rearrange("(m p) c -> p m c", p=P),
        )
        wo = [wo_all[:, m * C:(m + 1) * C] for m in range(NCT)]

        # persistent activation tiles
        # xT_all[:, cc*T + t] = x^T tile cc: [c-within-tile, t]
        xT_all = persist.tile([P, NCT * T], BF16, tag="xT", name="xT_all")
        qkT = [persist.tile([P, T], BF16, tag=f"qk{m}", name=f"qk{m}")
               for m in range(NCT)]
        # kT2[m]: rows 0:64 = k^T head 2m, rows 64:128 = k^T head 2m+1
        kT2 = [persist.tile([P, T], BF16, tag=f"k2{m}", name=f"k2{m}")
               for m in range(NCT)]
        vaug = [persist.tile([P, VAW], BF16, tag=f"va{i}", name=f"va{i}")
                for i in range(NT)]
        for i in range(NT):
            nc.gpsimd.memset(vaug[i][:], 1.0)  # ones column (+padding) preset
        anorm = [persist.tile([P, T], BF16, tag=f"an{m}", name=f"an{m}")
                 for m in range(NCT)]

        # ================= prologue =================
        # HAM warm-up: real (non-transpose) matmuls on junk data while the x
        # DMA lands, so the PE clock is at 8/8 when the real work starts.
        with tc.tile_pool(name="ps_warm", bufs=1, space="PSUM") as ps_warm:
            ps_w = ps_warm.tile([P, CHUNK], F32, tag="w", name="ps_w")
            for _ in range(5):
                nc.tensor.matmul(ps_w[:], identity[:], warm_rhs[:],
                                 start=True, stop=True)

        # exp ACT-table preload: a tiny dummy exp during the DMA wait pays the
        # ~2.7us one-time table-load cost before the real pipeline needs it.
        scratch16 = workp.tile([1, 16], F32, tag="scr16", name="scratch16")
        nc.scalar.activation(
            scratch16[:], warm_rhs[0:1, 0:16],
            mybir.ActivationFunctionType.Exp, scale=1.0)



        pv_cm = tc.tile_pool(name="ps_pv", bufs=1, space="PSUM", side="right")
        state_pv_pool = pv_cm.__enter__()
        tr_cm = tc.tile_pool(name="ps_tr", bufs=2, space="PSUM", side="right")
        tr_pool = tr_cm.__enter__()

        def emit_transpose(i):
            # x PE transpose; xT_all[:, cc*T + i*128 : ...] gets tile (i, cc)
            ps_tr = tr_pool.tile([P, C], BF16, tag="tr", name="ps_tr")
            for cc in range(NCT):
                nc.tensor.transpose(
                    ps_tr[:, cc * P:(cc + 1) * P],
                    x_in[:, i * C + cc * P: i * C + (cc + 1) * P],
                    identity[:],
                )
            # one strided copy: dest [128, cc, 128] with cc-stride T.
            # Tiles 4-7 copy on the (still idle) ScalarE to unload DVE.
            dst = xT_all[:].rearrange("p (cc t) -> p cc t", cc=NCT)
            if i < NT // 2:
                nc.vector.tensor_copy(
                    dst[:, :, i * P:(i + 1) * P],
                    ps_tr[:].rearrange("p (cc q) -> p cc q", cc=NCT),
                )
            else:
                nc.scalar.copy(
                    dst[:, :, i * P:(i + 1) * P],
                    ps_tr[:].rearrange("p (cc q) -> p cc q", cc=NCT),
                )

        qk_tag = [0]

        def qk_psum():
            # rotate across all four pv banks: they are idle until pair 1,
            # and a 4-deep rotation hides the DVE bias-add WAR latency
            qk_tag[0] = (qk_tag[0] + 1) % 4
            return state_pv_pool.tile(
                [P, CHUNK], F32, tag=f"pv{qk_tag[0]}", name="ps_qk")

        def emit_qk(m, j):
            ps_qk = qk_psum()
            js = slice(j * CHUNK, (j + 1) * CHUNK)
            for cc in range(NCT):
                if m == 0:
                    lhsT = wq_eq[:, cc * P:(cc + 1) * P]
                elif m == NCT:
                    lhsT = wq_ek[:, cc * P:(cc + 1) * P]
                else:
                    lhsT = wq[cc][:, m * P:(m + 1) * P]
                nc.tensor.matmul(
                    ps_qk[:],
                    lhsT,
                    xT_all[:, cc * T + j * CHUNK: cc * T + (j + 1) * CHUNK],
                    start=(cc == 0),
                    stop=(cc == NCT - 1),
                )
            dstt = qkT[m] if m < NCT else kT2[m - NCT]
            nc.vector.tensor_scalar_add(dstt[:, js], ps_qk[:], qkb_t[m][:])

        def emit_v(i):
            ps_v = qk_psum()
            for cc in range(NCT):
                nc.tensor.matmul(
                    ps_v[:],
                    xT_all[:, cc * T + i * P: cc * T + (i + 1) * P],
                    wq[cc][:, 2 * C:3 * C],
                    start=(cc == 0),
                    stop=(cc == NCT - 1),
                )
            va3 = vaug[i][:, 0:HEADS * VSTRIDE].rearrange(
                "p (h d) -> p h d", d=VSTRIDE)
            nc.vector.tensor_copy(
                va3[:, :, 0:HC],
                ps_v[:].rearrange("p (h d) -> p h d", h=HEADS),
            )

        # prologue part 2: transposes interleaved with pair 0's q/k tiles so
        # the first S^T slot is reachable as early as possible.  Chunk j of
        # qkT[0]/kT2[0] needs x tiles 4j..4j+3 transposed.
        for i in range(NT // 2):
            emit_transpose(i)
        emit_qk(0, 0)
        emit_qk(NCT, 0)
        for i in range(NT // 2, NT):
            emit_transpose(i)
        emit_qk(0, 1)
        emit_qk(NCT, 1)
        tr_cm.__exit__(None, None, None)

        # filler iterator: remaining phase-1 work, just-in-time order --
        # pair 1 needs (m1,k1) + all of v (for PV of pair 0); m2/k2 and
        # m3/k3 can wait until pairs 1-2, lightening pair 0's slots
        def phase1_fillers():
            for m in (1, NCT + 1):
                for j in range(NCH):
                    yield ("qk", m, j)
            for i in range(NT):
                yield ("v", i)
            for m in (2, NCT + 2, 3, NCT + 3):
                for j in range(NCH):
                    yield ("qk", m, j)

        fillers = phase1_fillers()
        state = {"fill_done": False, "tail_boxes": []}

        def run_fillers(n):
            for _ in range(n):
                try:
                    f = next(fillers)
                except StopIteration:
                    state["fill_done"] = True
                    return
                if f[0] == "qk":
                    emit_qk(f[1], f[2])
                else:
                    emit_v(f[1])

        # ================= phase 2: attention =================
        def emit_pv_chain(h, j, ppv, exh):
            base = (h % 2) * T
            for ssi in range(NT):
                nc.tensor.matmul(
                    ppv[:],
                    vaug[ssi][:, h * VSTRIDE: h * VSTRIDE + P],
                    exh[:, ssi * 2 * T + base + j * CHUNK:
                        ssi * 2 * T + base + (j + 1) * CHUNK],
                    start=(ssi == 0),
                    stop=(ssi == NT - 1),
                )

        def emit_recip(h, box, pe_pool=None):
            hh = h % 2
            dtmp = workp.tile([1, T], F32, tag=f"dtmp{hh}", name="dtmp")
            nc.vector.tensor_copy(dtmp[:, 0:CHUNK], box["p0"][HC:HC + 1, :])
            nc.vector.tensor_copy(dtmp[:, CHUNK:T], box["p1"][HC:HC + 1, :])
            recip = workp.tile([1, T], F32, tag=f"recip{hh}", name="recip")
            nc.vector.reciprocal_approx_fast(recip[:], dtmp[:])
            bcast = workp.tile([HC, T], F32, tag=f"bcast{hh}", name="bcast")
            if pe_pool is None:
                nc.gpsimd.partition_broadcast(bcast[:], recip[:], channels=HC)
            else:
                # PE broadcast + DVE copy: keeps GPSIMD (and its drains) off
                # the tail critical path, and the matmuls keep HAM warm
                bc_ps = pe_pool.tile([HC, T], F32, tag="bc", name="bc_ps")
                for j in range(NCH):
                    js = slice(j * CHUNK, (j + 1) * CHUNK)
                    nc.tensor.matmul(bc_ps[0:HC, js], ones1[:, 0:HC],
                                     recip[:, js], start=True, stop=True)
                nc.vector.tensor_copy(bcast[:], bc_ps[:])
            box["bc"] = bcast

        def emit_mults(h, box, js=(0, 1)):
            m = h // 2
            rlo = (h % 2) * HC
            for j in js:
                ppv = box["p0"] if j == 0 else box["p1"]
                nc.vector.tensor_tensor(
                    anorm[m][rlo:rlo + HC, j * CHUNK:(j + 1) * CHUNK],
                    ppv[0:HC, :],
                    box["bc"][:, j * CHUNK:(j + 1) * CHUNK],
                    op=mybir.AluOpType.mult,
                )

        def emit_pv_quarter(h, box, exh, lo):
            # 2 s-tiles x both chunks; the j0/j1 chains interleave so each
            # vaug-window weight load serves two adjacent matmuls
            base = (h % 2) * T
            for ssi in range(lo, lo + 2):
                for j, key in ((0, "p0"), (1, "p1")):
                    nc.tensor.matmul(
                        box[key][:],
                        vaug[ssi][:, h * VSTRIDE: h * VSTRIDE + P],
                        exh[:, ssi * 2 * T + base + j * CHUNK:
                            ssi * 2 * T + base + (j + 1) * CHUNK],
                        start=(ssi == 0),
                        stop=(ssi == NT - 1),
                    )

        def make_pv_steps(p, exh, tail=False):
            """Twelve ~0.9us payload units per pair: per head, each PV chunk
            chain is split into two 4-matmul halves plus a reciprocal step;
            both heads' normalize-multiplies come last (recips overlap)."""
            steps = []
            tail_mults = []
            for hh in range(2):
                h = 2 * p + hh
                box = {}

                def s_q0(h=h, hh=hh, box=box):
                    box["p0"] = state_pv_pool.tile(
                        [P, CHUNK], F32, tag=f"pv{2 * hh}", name="ppv0")
                    box["p1"] = state_pv_pool.tile(
                        [P, CHUNK], F32, tag=f"pv{2 * hh + 1}", name="ppv1")
                    emit_pv_quarter(h, box, exh, 0)

                def s_q1(h=h, box=box):
                    emit_pv_quarter(h, box, exh, 2)

                def s_q2(h=h, box=box):
                    emit_pv_quarter(h, box, exh, 4)

                def s_q3(h=h, box=box):
                    emit_pv_quarter(h, box, exh, 6)

                def s_recip(h=h, box=box):
                    emit_recip(h, box)

                def s_mults(h=h, box=box):
                    emit_mults(h, box)

                steps += [s_q0, s_q1, s_q2, s_q3, s_recip]
                if tail:
                    state["tail_boxes"].append(box)
                else:
                    tail_mults.append(s_mults)
            steps += tail_mults
            return steps

        exh_pool = ctx.enter_context(tc.tile_pool(name="exh", bufs=3))

        # right stack: opened after ps_tr closed, outlives the (left) qk pool
        st_cm = tc.tile_pool(name="ps_st", bufs=2, space="PSUM", side="right")
        st_pool = st_cm.__enter__()

        slots = [(p, ssi) for p in range(NPAIR) for ssi in range(NT)]
        exhs = []
        st_q = []

        def ensure_exh(p):
            while len(exhs) <= p:
                exhs.append(exh_pool.tile(
                    [P, NT * 2 * T], BF16, tag="exh", name="exh"))

        def emit_st(p, ssi):
            ensure_exh(p)
            sta = st_pool.tile([P, T], F32, tag="st", name="sta")
            stb = st_pool.tile([P, T], F32, tag="st", name="stb")
            for st_t, rlo in ((sta, 0), (stb, HC)):
                for j in range(NCH):
                    js = slice(j * CHUNK, (j + 1) * CHUNK)
                    nc.tensor.matmul(
                        st_t[:, js],
                        kT2[p][rlo:rlo + HC, ssi * P:(ssi + 1) * P],
                        qkT[p][rlo:rlo + HC, js],
                        start=True, stop=True,
                    )
            st_q.append((sta, stb))

        pending = []  # queue of PV/normalize closures for the previous pair
        emit_st(*slots[0])
        for g, (p, ssi) in enumerate(slots):
            exh = exhs[p]
            sta, stb = st_q.pop(0)
            nc.scalar.activation(
                exh[:, ssi * 2 * T: ssi * 2 * T + T],
                sta[:],
                mybir.ActivationFunctionType.Exp,
                scale=EXP_SCALE,
            )
            nc.scalar.activation(
                exh[:, ssi * 2 * T + T: (ssi + 1) * 2 * T],
                stb[:],
                mybir.ActivationFunctionType.Exp,
                scale=EXP_SCALE,
            )
            # next slot's S^T goes in front of this slot's payload work
            if g + 1 < len(slots):
                emit_st(*slots[g + 1])
            if ssi == 0 and p > 0:
                pending.extend(make_pv_steps(p - 1, exhs[p - 1]))
                if debug_dumps and p == 1:
                    nc.sync.dma_start(
                        dbg["dbg_exh0"].ap(), exhs[0][:, 0:2 * T])
            if ssi == 4 and p == NPAIR - 1:
                # pair 3's own PV joins the payload queue (its early-s-tile
                # halves only need already-finished exps); keeps the PE warm
                # through the last slots and shrinks the tail
                pending.extend(make_pv_steps(p, exh, tail=True))
            # PE-gap payloads for this slot
            if not state["fill_done"]:
                run_fillers(2)
            if state["fill_done"]:
                npop = 2 if len(pending) >= 6 else 1
                for _ in range(npop):
                    if pending:
                        pending.pop(0)()
        # drain everything (pair 3's normalize is handled below, woven
        # into the out-projection)
        while pending:
            pending.pop(0)()
        st_cm.__exit__(None, None, None)

        if debug_dumps:
            nc.sync.dma_start(dbg["dbg_xT"].ap(), xT_all[:])
            nc.sync.dma_start(dbg["dbg_q0"].ap(), qkT[0][:])
            nc.sync.dma_start(dbg["dbg_k0"].ap(), kT2[0][:])
            nc.sync.dma_start(dbg["dbg_va0"].ap(), vaug[0][:])

        # ================= phase 3: out projection =================
        # pair 3's normalize runs interleaved: PE-broadcast recips, then the
        # j0 mults (which unblock t-tiles 0..3), then the j1 mults.
        ps_o_cm = tc.tile_pool(name="ps_o", bufs=3, space="PSUM")
        ps_op = ps_o_cm.__enter__()
        box_a, box_b = state["tail_boxes"]
        otiles = []
        for i in range(3):
            ps_o = ps_op.tile([P, C], F32, tag="o", name="ps_o")
            otiles.append(ps_o)
            for cc in range(NCT - 1):
                nc.tensor.matmul(
                    ps_o[:],
                    anorm[cc][:, i * P:(i + 1) * P],
                    wo[cc][:],
                    start=(cc == 0),
                    stop=False,
                )
        emit_mults(2 * (NPAIR - 1), box_a, js=(0,))
        emit_mults(2 * (NPAIR - 1) + 1, box_b, js=(0,))
        emit_mults(2 * (NPAIR - 1), box_a, js=(1,))
        emit_mults(2 * (NPAIR - 1) + 1, box_b, js=(1,))

        def finish_tile(i, ps_o):
            nc.tensor.matmul(
                ps_o[:],
                anorm[NCT - 1][:, i * P:(i + 1) * P],
                wo[NCT - 1][:],
                start=False,
                stop=True,
            )
            osb = workp.tile([P, C], BF16, tag=f"osb{i}", name=f"osb{i}")
            nc.vector.tensor_tensor(
                osb[:], ps_o[:], ob_bcast[:], op=mybir.AluOpType.add)
            nc.sync.dma_start(out_d.ap()[i * P:(i + 1) * P, :], osb[:])

        for i in range(3):
            finish_tile(i, otiles[i])
        for i in range(3, NT):
            ps_o = ps_op.tile([P, C], F32, tag="o", name="ps_o")
            for cc in range(NCT - 1):
                nc.tensor.matmul(
                    ps_o[:],
                    anorm[cc][:, i * P:(i + 1) * P],
                    wo[cc][:],
                    start=(cc == 0),
                    stop=False,
                )
            finish_tile(i, ps_o)
        ps_o_cm.__exit__(None, None, None)
        pv_cm.__exit__(None, None, None)
        if debug_dumps:
            nc.sync.dma_start(dbg["dbg_an0"].ap(), anorm[0][:])

    nc.compile()
    return nc


_CACHED_NC = None


def _get_nc():
    global _CACHED_NC
    if _CACHED_NC is None:
        _CACHED_NC = build_program(
            debug_dumps=bool(int(os.environ.get("KERNEL_DEBUG", "0"))))
    return _CACHED_NC


def _prep_inputs(x, qkv_w, qkv_b, out_w, out_b):
    import ml_dtypes

    x = np.asarray(x)
    B = x.shape[0]
    x2 = x.reshape(B, T, C).astype(ml_dtypes.bfloat16)
    wq2 = np.asarray(qkv_w).reshape(C, 3 * C).astype(ml_dtypes.bfloat16)
    wo2 = np.asarray(out_w).reshape(C, C).astype(ml_dtypes.bfloat16)
    qkv_b = np.asarray(qkv_b).astype(np.float32)
    out_b = np.asarray(out_b).astype(np.float32)
    # fold the v-bias through the output projection (exact: A_norm += b_v
    # shifts out by b_v @ W_out since softmax rows sum to 1).
    b_v = qkv_b[2 * C:3 * C]
    ob_eff = (
        out_b.astype(np.float64)
        + b_v.astype(np.float64) @ wo2.astype(np.float64)
    ).astype(np.float32)
    qkb = np.ascontiguousarray(qkv_b[0:2 * C])
    return x2, wq2, wo2, qkb, ob_eff


def kernel(x, qkv_w, qkv_b, out_w, out_b):
    """Full inputs in, full output out.  Shards batch across 8 NeuronCores."""
    from concourse.bass_utils import run_bass_kernel_spmd

    x = np.asarray(x)
    B, H, W, Cc = x.shape
    assert (B, H, W, Cc) == (8, 32, 32, C)
    x2, wq2, wo2, qkb, ob_eff = _prep_inputs(x, qkv_w, qkv_b, out_w, out_b)

    nc = _get_nc()
    in_maps = [
        {
            "x": np.ascontiguousarray(x2[b]),
            "qkv_w": np.ascontiguousarray(wq2),
            "out_w": np.ascontiguousarray(wo2),
            "qk_b": qkb,
            "out_b": ob_eff,
        }
        for b in range(B)
    ]
    trace = bool(int(os.environ.get("KERNEL_TRACE", "0")))
    res = run_bass_kernel_spmd(nc, in_maps, core_ids=list(range(B)), trace=trace)
    if trace and res.exec_time_ns is not None:
        print(f"HW exec time: {res.exec_time_ns} ns")
    kernel.last_results = res
    out = np.stack(
        [np.asarray(res.results[b]["out"]).astype(np.float32) for b in range(B)],
        axis=0,
    )
    return out.reshape(B, H, W, Cc)


kernel.last_results = None
